# revision 26
# baseline (speedup 1.0000x reference)
"""Trainium2 Bass kernel for nn_Conv2DMod (StyleGAN2-style modulated 3x3 conv).

Problem: x[8,64,256,256], s[8,64], weight[64,64,3,3] (f32)
  w = weight * (s+1) per sample; demod by rsqrt(sum w^2 over (Cin,K,K));
  out[b] = conv2d(x[b], w_b, pad=1).

Sharding: data-parallel over batch. 8 samples -> 8 NeuronCores, one each.

Per-core algorithm (V6):
  - x is padded (H+2, W+2) and cast to bf16 on the HOST; device loads are
    plain HWDGE DMAs (sync + scalar queues) with contiguous multi-KB
    descriptors. The base weight is transposed to lhsT layout
    [Cin, (pos, Cout)] and cast to bf16 on the host (layout/cast only --
    all modulation math stays on device).
  - device weight prep critical path is two ops: w2 = wT * (s+1)
    (per-partition tensor_scalar) + one ACT copy to the high partition
    half. Demodulation is applied as a per-partition SCALE during psum
    evacuation; d = 1/sqrt(sum(w2^2)+eps) is computed via DVE square + 9
    tiny accumulating f32 matmuls against a ones-vector (partition-dim
    reduce), slotted between the first two conv t-steps so it doesn't
    delay the conv start.
  - conv as shift-matmul over 9 kernel positions on 4 independent 64x64 PE
    cells: row tile = block (xt partitions 0-63 = 34-row window for output
    rows [64i,64i+32), partitions 64-127 for [64i+32,64i+64)); col tile =
    which 16-row half of the block. Each cell accumulates ALL 9 positions
    for its own 2-output-row chunk into its own psum half-bank:
      bankX[0:64] = block0 rows (2t,2t+1)  bankX[64:128] = block0 rows (16+2t,..)
      bankY[0:64] = block1 rows (2t,2t+1)  bankY[64:128] = block1 rows (16+2t,..)
    so evacuation is one scaled [128,512] f32->bf16 copy per bank (ACT for
    bankX, DVE for bankY). Steady state runs at the PE streaming limit
    (215 ns per 4-matmul wave).
  - x pieces per block window interleave the two 17-row bands so t-step 0
    only needs ~0.4 MB; pieces alternate between the two HWDGE queues.
    Output flushes every 4 t-steps as [64, 2048] slices (4 KB contiguous
    per channel -- large packets keep HWDGE queue throughput high), on the
    same two queues. Output is bf16 [Cout, H, W]; host upcasts.
"""

import numpy as np
import ml_dtypes

import concourse.bacc as bacc
import concourse.mybir as mybir
import concourse.tile as tile
from concourse.bass import ts
from concourse.bass_utils import run_bass_kernel_spmd

F32 = mybir.dt.float32
BF16 = mybir.dt.bfloat16

B, CIN, COUT, KK, H, W = 8, 64, 64, 3, 256, 256
EPS = 1e-8
PH, PW = H + 2, W + 2   # host-padded input dims
HB = 32                 # output rows per block
NBI = H // (2 * HB)     # pair-iterations (4): block0+block1 = 64 rows each
NT = 8                  # t-steps per iteration; 4 chunks (2 rows) per step
XR = 2 * HB + 2         # xt rows per block window (34)
# x row pieces interleave the low band (rows 0..15+, read by col-tile 0)
# and high band (rows 16..33, col-tile 1) so early t-steps land first.
# i=0 streams fine-grained (compute consumes pieces as they land); later
# iterations are prefetched a full iteration ahead with 17-row pieces
# (8.8KB contiguous runs -> large packets -> ~2x queue throughput).
XPIECES_FINE = ((0, 6), (16, 22), (6, 10), (22, 26),
                (10, 14), (26, 30), (14, 16), (30, 34))
XPIECES_BIG = ((0, 17), (17, 34))


def build_nc():
    nc = bacc.Bacc("TRN2")
    x = nc.dram_tensor("x", [CIN, PH, PW], BF16, kind="ExternalInput")
    s = nc.dram_tensor("s", [CIN, 1], F32, kind="ExternalInput")
    wgt = nc.dram_tensor("wgt", [CIN, 9 * COUT], BF16, kind="ExternalInput")
    out = nc.dram_tensor("out", [COUT, H, W], BF16, kind="ExternalOutput")

    with tile.TileContext(nc) as tc:
        with (
            tc.tile_pool(name="const", bufs=1) as constp,
            tc.tile_pool(name="xpool", bufs=3) as xpool,
            tc.tile_pool(name="stpool", bufs=3) as stpool,
            tc.tile_pool(name="pspool", bufs=3, space="PSUM") as pspool,
            tc.tile_pool(name="dpool", bufs=1, space="PSUM") as dpool,
            tc.tile_pool(name="prepp", bufs=1) as prepp,
        ):
            # weights + s on the gpsimd (SWDGE) queue: contiguous
            # per-partition patterns; keeps both HWDGE queues free for x.
            wT = constp.tile([64, 9 * 64], BF16)
            nc.gpsimd.dma_start(out=wT[:, :], in_=wgt[:, :])
            s1 = constp.tile([64, 1], F32)
            nc.gpsimd.dma_start(out=s1[:, :], in_=s[:, :])
            w2 = constp.tile([128, 9 * 64], BF16)
            d2 = constp.tile([128, 1], F32)
            ones = constp.tile([64, 1], F32)
            nc.vector.memset(ones[:, :], 1.0)
            scr = constp.tile([128, 512], F32)
            nc.vector.memset(scr[:, :], 0.0)

            # x for i=0: issued before anything else on the HWDGE queues
            xts = [xpool.tile([128, XR, PW], BF16, name=f"xt{i}", tag="xt")
                   for i in range(1)]

            def load_x(i, xt):
                # block0 window: padded rows [64i, 64i+34) -> partitions 0-63
                # block1 window: padded rows [64i+32, 64i+66) -> parts 64-127
                pieces = XPIECES_FINE if i == 0 else XPIECES_BIG
                for k, (r0, r1) in enumerate(pieces):
                    qa, qb = ((nc.sync, nc.scalar),
                              (nc.scalar, nc.sync))[k % 2]
                    qa.dma_start(out=xt[0:64, r0:r1, :],
                                 in_=x[:, 64 * i + r0:64 * i + r1, :])
                    qb.dma_start(out=xt[64:128, r0:r1, :],
                                 in_=x[:, 64 * i + HB + r0:64 * i + HB + r1, :])

            load_x(0, xts[0])

            # PE HAM warmup on scratch data (results never read): fp32
            # matmuls run 4 cycles/row, so a few of them span the whole
            # window until x arrives, keeping the clock-gate at 8/8 with no
            # idle gap before the conv starts
            ps_w = dpool.tile([128, 512], F32, name="ps_warm", tag="pw")
            for k in range(3):
                nc.tensor.matmul(ps_w[:, :], scr[:, 0:128], scr[:, :],
                                 start=(k == 0), stop=(k == 2),
                                 skip_group_check=True)

            # ---- modulated weights (critical path: 2 ops) ----
            nc.vector.tensor_scalar_add(s1[:, :], s1[:, :], 1.0)
            nc.vector.tensor_scalar_mul(w2[0:64, :], wT[:, :], s1[:, :])
            nc.scalar.activation(w2[64:128, :], w2[0:64, :],
                                 mybir.ActivationFunctionType.Copy)
            # demod-scale ingredients (matmuls issued inside the i=0/t=0 slot)
            sq = prepp.tile([64, 9 * 64], F32)
            nc.vector.tensor_mul(sq[:, :], w2[0:64, :], w2[0:64, :])
            epst = prepp.tile([64, 1], F32)
            nc.vector.memset(epst[:, :], EPS)
            dtmp = prepp.tile([64, 1], F32)
            d_col = prepp.tile([64, 1], F32)

            # ---- main conv loop ----
            for i in range(NBI):
                xt = xts[i]
                # prefetch next iteration's x before any flush submits can
                # block the HWDGE queues
                if i + 1 < NBI:
                    xts.append(xpool.tile([128, XR, PW], BF16,
                                          name=f"xt{i + 1}", tag="xt"))
                    load_x(i + 1, xts[i + 1])

                stg0 = stpool.tile([128, NT, 512], BF16,
                                   name=f"stg0_{i}", tag="stg0")
                stg1 = stpool.tile([128, NT, 512], BF16,
                                   name=f"stg1_{i}", tag="stg1")
                for t in range(NT):
                    bx = pspool.tile([128, 2, 256], F32,
                                     name=f"bx{i}_{t}", tag="bx")
                    by = pspool.tile([128, 2, 256], F32,
                                     name=f"by{i}_{t}", tag="by")
                    for p in range(9):
                        dy, dx = divmod(p, 3)
                        wlo = w2[0:64, ts(p, 64)]
                        whi = w2[64:128, ts(p, 64)]
                        st = dict(start=(p == 0), stop=(p == 8))
                        ra = 2 * t + dy          # col-tile-0 chunk rows
                        rb = 16 + 2 * t + dy     # col-tile-1 chunk rows
                        nc.tensor.matmul(
                            bx[0:64, :, :], wlo,
                            xt[0:64, ra:ra + 2, dx:dx + W],
                            tile_position=(0, 0), **st)
                        nc.tensor.matmul(
                            by[0:64, :, :], whi,
                            xt[64:128, ra:ra + 2, dx:dx + W],
                            tile_position=(64, 0), **st)
                        nc.tensor.matmul(
                            bx[64:128, :, :], wlo,
                            xt[0:64, rb:rb + 2, dx:dx + W],
                            tile_position=(0, 64), **st)
                        nc.tensor.matmul(
                            by[64:128, :, :], whi,
                            xt[64:128, rb:rb + 2, dx:dx + W],
                            tile_position=(64, 64), **st)

                    if i == 0 and t == 0:
                        # d = 1/sqrt(sum w2^2 + eps): 9 accumulating f32
                        # matmuls reduce over the Cin partition dim; runs on
                        # the PE right after the first conv t-step.
                        d_ps = dpool.tile([64, 1], F32, name="d_ps", tag="dps")
                        for p in range(9):
                            nc.tensor.matmul(d_ps[:, :], sq[:, ts(p, 64)],
                                             ones[:, :],
                                             start=(p == 0), stop=(p == 8))
                        nc.scalar.activation(dtmp[:, :], d_ps[:, :],
                                             mybir.ActivationFunctionType.Sqrt,
                                             bias=epst[:, :])
                        nc.vector.reciprocal(d_col[:, :], dtmp[:, :])
                        nc.vector.tensor_copy(d2[0:64, :], d_col[:, :])
                        nc.vector.tensor_copy(d2[64:128, :], d_col[:, :])

                    # evacuate with demod scale: ACT + DVE, one bank each
                    nc.scalar.activation(stg0[:, t, :], bx[:, :, :],
                                         mybir.ActivationFunctionType.Copy,
                                         scale=d2[:, :])
                    nc.vector.tensor_scalar_mul(stg1[:, t, :], by[:, :, :],
                                                d2[:, :])
                    # flush points: every 4 t-steps (4KB packets); the last
                    # iteration flushes finest to shrink the kernel tail.
                    # Mid-kernel, block1 flushes ride the otherwise-idle
                    # SWDGE queue (stage bufs=3 absorbs its latency); the
                    # last iteration stays on HWDGE.
                    if i == NBI - 1:
                        flush_pts = ((3, 0, 4), (6, 4, 3), (7, 7, 1))
                        q0, q1 = nc.sync, nc.scalar
                    else:
                        flush_pts = ((1, 0, 2), (3, 2, 2), (5, 4, 2), (7, 6, 2))
                        q0 = nc.sync if i % 2 == 0 else nc.scalar
                        q1 = nc.gpsimd
                    for ft, t0, nslc in flush_pts:
                        if t != ft:
                            continue
                        tsl = slice(t0, t0 + nslc)
                        r0 = 64 * i + 2 * t0
                        nr = 2 * nslc
                        q0.dma_start(
                            out=out[:, r0:r0 + nr, :],
                            in_=stg0[0:64, tsl, :])
                        q0.dma_start(
                            out=out[:, r0 + 16:r0 + 16 + nr, :],
                            in_=stg0[64:128, tsl, :])
                        q1.dma_start(
                            out=out[:, r0 + 32:r0 + 32 + nr, :],
                            in_=stg1[0:64, tsl, :])
                        q1.dma_start(
                            out=out[:, r0 + 48:r0 + 48 + nr, :],
                            in_=stg1[64:128, tsl, :])
    nc.finalize()
    return nc


_NC = None


def _get_nc():
    global _NC
    if _NC is None:
        _NC = build_nc()
    return _NC


def make_in_maps(x, s, weight):
    x = np.asarray(x, dtype=np.float32)
    s = np.ascontiguousarray(np.asarray(s, dtype=np.float32))
    # [o, i, kh, kw] -> [i, (kh, kw), o] lhsT layout, bf16 (layout/cast only)
    wT = np.ascontiguousarray(
        np.asarray(weight, dtype=np.float32).transpose(1, 2, 3, 0)
        .reshape(CIN, 9 * COUT)).astype(ml_dtypes.bfloat16)
    xp = np.zeros((B, CIN, PH, PW), dtype=ml_dtypes.bfloat16)
    xp[:, :, 1:PH - 1, 1:PW - 1] = x
    return [
        {"x": xp[c], "s": s[c].reshape(CIN, 1), "wgt": wT}
        for c in range(B)
    ]


def run(x, s, weight, **kw):
    nc = _get_nc()
    res = run_bass_kernel_spmd(nc, make_in_maps(x, s, weight),
                               core_ids=list(range(B)), **kw)
    out = np.stack([np.asarray(r["out"]) for r in res.results])
    return out.astype(np.float32), res


def kernel(x, s, weight):
    out, _ = run(x, s, weight)
    return out


if __name__ == "__main__":
    rng = np.random.default_rng(0)
    xv = rng.standard_normal((B, CIN, H, W), dtype=np.float32)
    sv = rng.standard_normal((B, CIN), dtype=np.float32)
    wv = (rng.standard_normal((COUT, CIN, KK, KK), dtype=np.float32)
          * np.float32(np.sqrt(2.0 / (CIN * KK * KK))))
    o = kernel(xv, sv, wv)
    print("ran ok", o.shape, o.dtype, float(np.abs(o).max()))


# revision 27
# speedup vs baseline: 1.0339x; 1.0339x over previous
"""Trainium2 Bass kernel for nn_Conv2DMod (StyleGAN2-style modulated 3x3 conv).

Problem: x[8,64,256,256], s[8,64], weight[64,64,3,3] (f32)
  w = weight * (s+1) per sample; demod by rsqrt(sum w^2 over (Cin,K,K));
  out[b] = conv2d(x[b], w_b, pad=1).

Sharding: data-parallel over batch. 8 samples -> 8 NeuronCores, one each.

Per-core algorithm (V6):
  - x is padded (H+2, W+2) and cast to bf16 on the HOST; device loads are
    plain HWDGE DMAs (sync + scalar queues) with contiguous multi-KB
    descriptors. The base weight is transposed to lhsT layout
    [Cin, (pos, Cout)] and cast to bf16 on the host (layout/cast only --
    all modulation math stays on device).
  - device weight prep critical path is two ops: w2 = wT * (s+1)
    (per-partition tensor_scalar) + one ACT copy to the high partition
    half. Demodulation is applied as a per-partition SCALE during psum
    evacuation; d = 1/sqrt(sum(w2^2)+eps) is computed via DVE square + 9
    tiny accumulating f32 matmuls against a ones-vector (partition-dim
    reduce), slotted between the first two conv t-steps so it doesn't
    delay the conv start.
  - conv as shift-matmul over 9 kernel positions on 4 independent 64x64 PE
    cells: row tile = block (xt partitions 0-63 = 34-row window for output
    rows [64i,64i+32), partitions 64-127 for [64i+32,64i+64)); col tile =
    which 16-row half of the block. Each cell accumulates ALL 9 positions
    for its own 2-output-row chunk into its own psum half-bank:
      bankX[0:64] = block0 rows (2t,2t+1)  bankX[64:128] = block0 rows (16+2t,..)
      bankY[0:64] = block1 rows (2t,2t+1)  bankY[64:128] = block1 rows (16+2t,..)
    so evacuation is one scaled [128,512] f32->bf16 copy per bank (ACT for
    bankX, DVE for bankY). Steady state runs at the PE streaming limit
    (215 ns per 4-matmul wave).
  - x pieces per block window interleave the two 17-row bands so t-step 0
    only needs ~0.4 MB; pieces alternate between the two HWDGE queues.
    Output flushes every 4 t-steps as [64, 2048] slices (4 KB contiguous
    per channel -- large packets keep HWDGE queue throughput high), on the
    same two queues. Output is bf16 [Cout, H, W]; host upcasts.
"""

import numpy as np
import ml_dtypes

import concourse.bacc as bacc
import concourse.mybir as mybir
import concourse.tile as tile
from concourse.bass import ts
from concourse.bass_utils import run_bass_kernel_spmd

F32 = mybir.dt.float32
BF16 = mybir.dt.bfloat16

B, CIN, COUT, KK, H, W = 8, 64, 64, 3, 256, 256
EPS = 1e-8
PH, PW = H + 2, W + 2   # host-padded input dims
HB = 32                 # output rows per block
NBI = H // (2 * HB)     # pair-iterations (4): block0+block1 = 64 rows each
NT = 8                  # t-steps per iteration; 4 chunks (2 rows) per step
XR = 2 * HB + 2         # xt rows per block window (34)
# x row pieces interleave the low band (rows 0..15+, read by col-tile 0)
# and high band (rows 16..33, col-tile 1) so early t-steps land first.
# i=0 streams fine-grained (compute consumes pieces as they land); later
# iterations are prefetched a full iteration ahead with 17-row pieces
# (8.8KB contiguous runs -> large packets -> ~2x queue throughput).
XPIECES_FINE = ((0, 6), (16, 22), (6, 10), (22, 26),
                (10, 14), (26, 30), (14, 16), (30, 34))
XPIECES_BIG = ((0, 17), (17, 34))


def build_nc():
    nc = bacc.Bacc("TRN2")
    x = nc.dram_tensor("x", [CIN, PH, PW], BF16, kind="ExternalInput")
    s = nc.dram_tensor("s", [CIN, 1], F32, kind="ExternalInput")
    wgt = nc.dram_tensor("wgt", [CIN, 9 * COUT], BF16, kind="ExternalInput")
    out = nc.dram_tensor("out", [COUT, H, W], BF16, kind="ExternalOutput")

    with tile.TileContext(nc) as tc:
        with (
            tc.tile_pool(name="const", bufs=1) as constp,
            tc.tile_pool(name="xpool", bufs=3) as xpool,
            tc.tile_pool(name="stpool", bufs=3) as stpool,
            tc.tile_pool(name="pspool", bufs=3, space="PSUM") as pspool,
            tc.tile_pool(name="dpool", bufs=1, space="PSUM") as dpool,
            tc.tile_pool(name="prepp", bufs=1) as prepp,
        ):
            # weights + s on the gpsimd (SWDGE) queue: contiguous
            # per-partition patterns; keeps both HWDGE queues free for x.
            wT = constp.tile([64, 9 * 64], BF16)
            nc.gpsimd.dma_start(out=wT[:, :], in_=wgt[:, :])
            s1 = constp.tile([64, 1], F32)
            nc.gpsimd.dma_start(out=s1[:, :], in_=s[:, :])
            w2 = constp.tile([128, 9 * 64], BF16)
            d2 = constp.tile([128, 1], F32)
            ones = constp.tile([64, 1], F32)
            nc.vector.memset(ones[:, :], 1.0)
            scr = constp.tile([128, 512], F32)
            nc.vector.memset(scr[:, :], 0.0)

            # x for i=0: issued before anything else on the HWDGE queues
            xts = [xpool.tile([128, XR, PW], BF16, name=f"xt{i}", tag="xt")
                   for i in range(1)]

            def load_x(i, xt):
                # block0 window: padded rows [64i, 64i+34) -> partitions 0-63
                # block1 window: padded rows [64i+32, 64i+66) -> parts 64-127
                pieces = XPIECES_FINE if i == 0 else XPIECES_BIG
                for k, (r0, r1) in enumerate(pieces):
                    qa, qb = ((nc.sync, nc.scalar),
                              (nc.scalar, nc.sync))[k % 2]
                    qa.dma_start(out=xt[0:64, r0:r1, :],
                                 in_=x[:, 64 * i + r0:64 * i + r1, :])
                    qb.dma_start(out=xt[64:128, r0:r1, :],
                                 in_=x[:, 64 * i + HB + r0:64 * i + HB + r1, :])

            load_x(0, xts[0])

            # PE HAM warmup on scratch data (results never read): fp32
            # matmuls run 4 cycles/row, so a few of them span the whole
            # window until x arrives, keeping the clock-gate at 8/8 with no
            # idle gap before the conv starts
            ps_w = dpool.tile([128, 512], F32, name="ps_warm", tag="pw")
            for k in range(3):
                nc.tensor.matmul(ps_w[:, :], scr[:, 0:128], scr[:, :],
                                 start=(k == 0), stop=(k == 2),
                                 skip_group_check=True)

            # ---- modulated weights (critical path: 2 ops) ----
            nc.vector.tensor_scalar_add(s1[:, :], s1[:, :], 1.0)
            nc.vector.tensor_scalar_mul(w2[0:64, :], wT[:, :], s1[:, :])
            nc.scalar.activation(w2[64:128, :], w2[0:64, :],
                                 mybir.ActivationFunctionType.Copy)
            # demod-scale ingredients (matmuls issued inside the i=0/t=0 slot)
            sq = prepp.tile([64, 9 * 64], F32)
            nc.vector.tensor_mul(sq[:, :], w2[0:64, :], w2[0:64, :])
            epst = prepp.tile([64, 1], F32)
            nc.vector.memset(epst[:, :], EPS)
            dtmp = prepp.tile([64, 1], F32)
            d_col = prepp.tile([64, 1], F32)

            # ---- main conv loop ----
            for i in range(NBI):
                xt = xts[i]
                # prefetch next iteration's x before any flush submits can
                # block the HWDGE queues
                if i + 1 < NBI:
                    xts.append(xpool.tile([128, XR, PW], BF16,
                                          name=f"xt{i + 1}", tag="xt"))
                    load_x(i + 1, xts[i + 1])

                stg0 = stpool.tile([128, NT, 512], BF16,
                                   name=f"stg0_{i}", tag="stg0")
                stg1 = stpool.tile([128, NT, 512], BF16,
                                   name=f"stg1_{i}", tag="stg1")
                for t in range(NT):
                    bx = pspool.tile([128, 2, 256], F32,
                                     name=f"bx{i}_{t}", tag="bx")
                    by = pspool.tile([128, 2, 256], F32,
                                     name=f"by{i}_{t}", tag="by")
                    for p in range(9):
                        dy, dx = divmod(p, 3)
                        wlo = w2[0:64, ts(p, 64)]
                        whi = w2[64:128, ts(p, 64)]
                        st = dict(start=(p == 0), stop=(p == 8))
                        ra = 2 * t + dy          # col-tile-0 chunk rows
                        rb = 16 + 2 * t + dy     # col-tile-1 chunk rows
                        nc.tensor.matmul(
                            bx[0:64, :, :], wlo,
                            xt[0:64, ra:ra + 2, dx:dx + W],
                            tile_position=(0, 0), **st)
                        nc.tensor.matmul(
                            by[0:64, :, :], whi,
                            xt[64:128, ra:ra + 2, dx:dx + W],
                            tile_position=(64, 0), **st)
                        nc.tensor.matmul(
                            bx[64:128, :, :], wlo,
                            xt[0:64, rb:rb + 2, dx:dx + W],
                            tile_position=(0, 64), **st)
                        nc.tensor.matmul(
                            by[64:128, :, :], whi,
                            xt[64:128, rb:rb + 2, dx:dx + W],
                            tile_position=(64, 64), **st)

                    if i == 0 and t == 0:
                        # d = 1/sqrt(sum w2^2 + eps): 9 accumulating f32
                        # matmuls reduce over the Cin partition dim; runs on
                        # the PE right after the first conv t-step.
                        d_ps = dpool.tile([64, 1], F32, name="d_ps", tag="dps")
                        for p in range(9):
                            nc.tensor.matmul(d_ps[:, :], sq[:, ts(p, 64)],
                                             ones[:, :],
                                             start=(p == 0), stop=(p == 8))
                        nc.scalar.activation(dtmp[:, :], d_ps[:, :],
                                             mybir.ActivationFunctionType.Sqrt,
                                             bias=epst[:, :])
                        nc.vector.reciprocal(d_col[:, :], dtmp[:, :])
                        nc.vector.tensor_copy(d2[0:64, :], d_col[:, :])
                        nc.vector.tensor_copy(d2[64:128, :], d_col[:, :])

                    # evacuate with demod scale: ACT + DVE, one bank each
                    nc.scalar.activation(stg0[:, t, :], bx[:, :, :],
                                         mybir.ActivationFunctionType.Copy,
                                         scale=d2[:, :])
                    nc.vector.tensor_scalar_mul(stg1[:, t, :], by[:, :, :],
                                                d2[:, :])
                    # flush points: every 4 t-steps (4KB packets); the last
                    # iteration flushes finest to shrink the kernel tail.
                    # Mid-kernel, block1 flushes ride the otherwise-idle
                    # SWDGE queue (stage bufs=3 absorbs its latency); the
                    # last iteration stays on HWDGE.
                    if i == NBI - 1:
                        flush_pts = ((3, 0, 4), (6, 4, 3), (7, 7, 1))
                        q0, q1 = nc.sync, nc.scalar
                    else:
                        flush_pts = ((3, 0, 4), (7, 4, 4))
                        q0 = nc.sync if i % 2 == 0 else nc.scalar
                        q1 = nc.gpsimd
                    for ft, t0, nslc in flush_pts:
                        if t != ft:
                            continue
                        tsl = slice(t0, t0 + nslc)
                        r0 = 64 * i + 2 * t0
                        nr = 2 * nslc
                        q0.dma_start(
                            out=out[:, r0:r0 + nr, :],
                            in_=stg0[0:64, tsl, :])
                        q0.dma_start(
                            out=out[:, r0 + 16:r0 + 16 + nr, :],
                            in_=stg0[64:128, tsl, :])
                        q1.dma_start(
                            out=out[:, r0 + 32:r0 + 32 + nr, :],
                            in_=stg1[0:64, tsl, :])
                        q1.dma_start(
                            out=out[:, r0 + 48:r0 + 48 + nr, :],
                            in_=stg1[64:128, tsl, :])
    nc.finalize()
    return nc


_NC = None


def _get_nc():
    global _NC
    if _NC is None:
        _NC = build_nc()
    return _NC


def make_in_maps(x, s, weight):
    x = np.asarray(x, dtype=np.float32)
    s = np.ascontiguousarray(np.asarray(s, dtype=np.float32))
    # [o, i, kh, kw] -> [i, (kh, kw), o] lhsT layout, bf16 (layout/cast only)
    wT = np.ascontiguousarray(
        np.asarray(weight, dtype=np.float32).transpose(1, 2, 3, 0)
        .reshape(CIN, 9 * COUT)).astype(ml_dtypes.bfloat16)
    xp = np.zeros((B, CIN, PH, PW), dtype=ml_dtypes.bfloat16)
    xp[:, :, 1:PH - 1, 1:PW - 1] = x
    return [
        {"x": xp[c], "s": s[c].reshape(CIN, 1), "wgt": wT}
        for c in range(B)
    ]


def run(x, s, weight, **kw):
    nc = _get_nc()
    res = run_bass_kernel_spmd(nc, make_in_maps(x, s, weight),
                               core_ids=list(range(B)), **kw)
    out = np.stack([np.asarray(r["out"]) for r in res.results])
    return out.astype(np.float32), res


def kernel(x, s, weight):
    out, _ = run(x, s, weight)
    return out


if __name__ == "__main__":
    rng = np.random.default_rng(0)
    xv = rng.standard_normal((B, CIN, H, W), dtype=np.float32)
    sv = rng.standard_normal((B, CIN), dtype=np.float32)
    wv = (rng.standard_normal((COUT, CIN, KK, KK), dtype=np.float32)
          * np.float32(np.sqrt(2.0 / (CIN * KK * KK))))
    o = kernel(xv, sv, wv)
    print("ran ok", o.shape, o.dtype, float(np.abs(o).max()))
